# revision 1
# baseline (speedup 1.0000x reference)
"""Trainium2 Bass kernel for nn_DHSFNN_6038724018534.

Two-layer dendritic-branch spiking net with leaky-integrator softmax readout.
Pure data parallel over batch: 8 cores x 32 batch rows each, weights replicated.

Per-core structure:
  Phase 0: cast x (fp32) -> bf16 DRAM copy, then DMA-xbar-transpose to get
           x^T tiles [IN-part, (b,t)-free] for the tensor engine.
  Phase A: c1 = W1'^T x^T (bf16 matmul, (1-beta1)(1-a1) folded into W1 rows,
           bias applied during PSUM->SBUF copy on ScalarE), then per-branch
           dendrite IIR via VectorE tensor_tensor_scan along t, branch-sum
           adds -> V1 (input drive of layer-1 membranes, all t).
  Phase B: serial membrane/spike scan over t with BOTH layers merged into
           [128,128] VectorE ops (layer 2 lagged 25 steps so its drive V2 is
           produced chunk-by-chunk from matmul2 of layer-1 spikes); spikes
           extracted as exact {0,1} bf16 for the next matmul.
  Readout: matmul3 -> fp32 bias path (br/ar kept fp32 end-to-end), strided
           mr IIR scans, exp on ScalarE, PE-transpose + row softmax, and a
           selector-matmul accumulation over t skipping the warmup steps.
"""

import numpy as np
import ml_dtypes

BF16 = ml_dtypes.bfloat16

# Problem shapes (hardcoded per the task contract).
B_FULL, T, IN, H, OUT, BR = 256, 250, 700, 256, 20, 4
NCORES = 8
B = B_FULL // NCORES          # 32 per core
ROWS = B * T                  # 8000 (b-major rows of x per core)
INP = 768                     # padded contraction dim (6 chunks of 128)
KCH = 6                       # k-chunks for matmul1 (last one has 60 rows)
HR = H * BR                   # 1024 rows of W1'/W2'
MCH = HR // 128               # 8 m-chunks
TC = 25                       # phase-B chunk length
NCHUNK = T // TC              # 10
LAG = TC                      # layer-2 lag in steps
TAU = T + LAG                 # 275 merged-scan steps / V time axis
QCOLS = 2000                  # phase-A quarter: 8 batch rows x 250 t
NQ = ROWS // QCOLS            # 4


def _sigmoid(v):
    return 1.0 / (1.0 + np.exp(-v.astype(np.float64)))


def _prep_constants(W1, b1, tau_n1, tau_m1, W2, b2, tau_n2, tau_m2, Wr, br, tau_mr):
    """Host-side packing of the (small) replicated weights."""
    beta1 = _sigmoid(tau_n1)          # [H, BR]
    a1 = _sigmoid(tau_m1)             # [H]
    beta2 = _sigmoid(tau_n2)
    a2 = _sigmoid(tau_m2)
    ar = _sigmoid(tau_mr)             # [OUT]

    # Row mapping m = j*H + h  ->  original row h*BR + j
    mj, mh = np.divmod(np.arange(HR), H)          # j = m//H, h = m%H
    orig = mh * BR + mj
    scale1 = (1.0 - beta1[mh, mj]) * (1.0 - a1[mh])     # [HR]
    scale2 = (1.0 - beta2[mh, mj]) * (1.0 - a2[mh])

    w1p = W1.astype(np.float64)[orig] * scale1[:, None]   # [HR, IN]
    w2p = W2.astype(np.float64)[orig] * scale2[:, None]   # [HR, H]
    b1p = b1.astype(np.float64)[orig] * scale1            # [HR]
    b2p = b2.astype(np.float64)[orig] * scale2

    # w1t[p, kc*HR + m] = w1p[m, kc*128+p], zero-padded beyond IN.
    w1pad = np.zeros((HR, INP), np.float64)
    w1pad[:, :IN] = w1p
    w1t = np.ascontiguousarray(
        w1pad.T.reshape(KCH, 128, HR).transpose(1, 0, 2).reshape(128, KCH * HR)
    ).astype(BF16)
    w2t = np.ascontiguousarray(
        w2p.T.reshape(2, 128, HR).transpose(1, 0, 2).reshape(128, 2 * HR)
    ).astype(BF16)

    b1c = np.ascontiguousarray(b1p.reshape(MCH, 128).T).astype(np.float32)  # [128, 8]
    b2c = np.ascontiguousarray(b2p.reshape(MCH, 128).T).astype(np.float32)

    # bz1[p, mc, t]: 0 at t=0 else beta1 of row m = mc*128+p
    beta1_m = beta1[mh, mj].reshape(MCH, 128).T          # [128, MCH]
    beta2_m = beta2[mh, mj].reshape(MCH, 128).T
    bz1 = np.repeat(beta1_m[:, :, None], T, axis=2)
    bz1[:, :, 0] = 0.0
    bz1 = bz1.astype(BF16)                                # [128, 8, 250]
    # bz2[p, mc, b*26+i]: 0 at i==0 (boundary col) else beta2
    bz2 = np.repeat(beta2_m[:, :, None], B * (TC + 1), axis=2).reshape(
        128, MCH, B, TC + 1
    )
    bz2[:, :, :, 0] = 0.0
    bz2 = bz2.reshape(128, MCH, B * (TC + 1)).astype(BF16)  # [128, 8, 832]

    # abc[p, (L, hh, b)] = a_L[hh*128 + p]
    abc = np.empty((128, 2, 2, B), np.float64)
    for hh in range(2):
        abc[:, 0, hh, :] = a1[hh * 128:(hh + 1) * 128, None]
        abc[:, 1, hh, :] = a2[hh * 128:(hh + 1) * 128, None]
    abc = abc.astype(BF16)

    wrp = Wr.astype(np.float64) * (1.0 - ar)[:, None]     # [OUT, H]
    wrt = np.ascontiguousarray(
        wrp.T.reshape(2, 128, OUT).transpose(1, 0, 2).reshape(128, 2 * OUT)
    ).astype(BF16)
    brc = (br.astype(np.float64) * (1.0 - ar)).astype(np.float32).reshape(OUT, 1)

    arz = np.repeat(ar[:, None], T, axis=1)
    arz[:, 0] = 0.0
    arz = arz.astype(np.float32)                          # [20, 250]

    esel = (np.arange(128)[:, None] % B == np.arange(B)[None, :]).astype(np.float32)
    idm = np.eye(OUT, dtype=np.float32)

    return dict(w1t=w1t, w2t=w2t, b1c=b1c, b2c=b2c, bz1=bz1, bz2=bz2,
                abc=abc, wrt=wrt, brc=brc, arz=arz, esel=esel, idm=idm)


def _build_program(num_devices=NCORES):
    from contextlib import ExitStack
    import concourse.bacc as bacc
    import concourse.tile as tile
    from concourse import mybir

    dt = mybir.dt
    Alu = mybir.AluOpType
    Act = mybir.ActivationFunctionType

    nc = bacc.Bacc("TRN2", target_bir_lowering=False, debug=False,
                   num_devices=num_devices)

    x = nc.dram_tensor("x", [ROWS, IN], dt.float32, kind="ExternalInput").ap()
    cons = {}
    for name, shape, d in [
        ("w1t", [128, KCH * HR], dt.bfloat16),
        ("w2t", [128, 2 * HR], dt.bfloat16),
        ("b1c", [128, MCH], dt.float32),
        ("b2c", [128, MCH], dt.float32),
        ("bz1", [128, MCH, T], dt.bfloat16),
        ("bz2", [128, MCH, B * (TC + 1)], dt.bfloat16),
        ("abc", [128, 2, 2, B], dt.bfloat16),
        ("wrt", [128, 2 * OUT], dt.bfloat16),
        ("brc", [OUT, 1], dt.float32),
        ("arz", [OUT, T], dt.float32),
        ("esel", [128, B], dt.float32),
        ("idm", [OUT, OUT], dt.float32),
    ]:
        cons[name] = nc.dram_tensor(name, shape, d, kind="ExternalInput").ap()
    out = nc.dram_tensor("out", [B, OUT], dt.float32, kind="ExternalOutput").ap()

    xbf = nc.dram_tensor("xbf", [ROWS, INP], dt.bfloat16).ap()

    with tile.TileContext(nc) as tc, ExitStack() as ctx:
        # ---- persistent SBUF ----
        cpool = ctx.enter_context(tc.tile_pool(name="consts", bufs=1))
        sb = {}

        def load_const(pool, name):
            ap = cons[name]
            t = pool.tile(list(ap.shape), ap.dtype, name=f"sb_{name}",
                          tag=f"sb_{name}")
            nc.gpsimd.dma_start(out=t[:], in_=ap[:])
            sb[name] = t

        load_const(cpool, "abc")

        gpool = ctx.enter_context(tc.tile_pool(name="glob", bufs=1))
        V = gpool.tile([128, 2, 2, B, TAU], dt.bfloat16)     # membrane drive
        S = gpool.tile([128, 2, 2, B, 2 * TC], dt.bfloat16)  # spike ring
        M = gpool.tile([128, 2, 2, B], dt.bfloat16)          # membrane state
        ZC = gpool.tile([128, 2, 2, B], dt.bfloat16)         # zeros
        D2K = gpool.tile([128, MCH, B, 1], dt.bfloat16)      # layer-2 scan boundary
        C3 = gpool.tile([OUT, B * T], dt.float32)            # readout drive (t-major)

        nc.vector.memset(M[:], 0.0)
        nc.vector.memset(ZC[:], 0.0)
        nc.vector.memset(D2K[:], 0.0)
        nc.vector.memset(V[:, 1, :, :, 0:LAG], 0.0)
        nc.vector.memset(V[:, 0, :, :, T:TAU], 0.0)

        # ---- phase 0: cast x -> xbf (bf16, row pitch 768) ----
        with tc.tile_pool(name="cast", bufs=3) as castp:
            CP, CR = 125, 4        # 16 blocks x 125 partitions x 4 rows
            xv = x.rearrange("(blk p r) c -> blk p (r c)", p=CP, r=CR)
            ov = xbf.rearrange("(blk p r) c -> blk p r c", p=CP, r=CR)
            for blk in range(ROWS // (CP * CR)):
                tf = castp.tile([CP, CR * IN], dt.float32, tag="castf")
                nc.gpsimd.dma_start(out=tf[:], in_=xv[blk])
                tb = castp.tile([CP, CR * IN], dt.bfloat16, tag="castb")
                nc.scalar.copy(tb[:], tf[:])
                nc.gpsimd.dma_start(
                    out=ov[blk, :, :, 0:IN],
                    in_=tb[:].rearrange("p (r c) -> p r c", r=CR),
                )

        # ---- phase A: layer-1 drive V1 ----
        with tc.tile_pool(name="cA", bufs=1) as cAp, \
             tc.tile_pool(name="xt", bufs=10) as xtp, \
             tc.tile_pool(name="c1", bufs=2) as c1p, \
             tc.tile_pool(name="dacc", bufs=1) as daccp, \
             tc.tile_pool(name="dtmp", bufs=2) as dtmpp, \
             tc.tile_pool(name="psA", bufs=2, space="PSUM") as psA:
            for name in ("w1t", "b1c", "bz1"):
                load_const(cAp, name)
            for q in range(NQ):
                xt = []
                for kc in range(KCH):
                    xtile = xtp.tile([128, QCOLS], dt.bfloat16, tag="xt")
                    nc.sync.dma_start(
                        out=xtile[:],
                        in_=xbf[q * QCOLS:(q + 1) * QCOLS, kc * 128:(kc + 1) * 128],
                        transpose=True,
                    )
                    xt.append(xtile)
                dacc = [[daccp.tile([128, QCOLS], dt.bfloat16,
                                     tag=f"dacc{hh}{j}", name=f"dacc{hh}{j}")
                         for j in range(3)] for hh in range(2)]
                for mc in range(MCH):
                    j, hh = divmod(mc, 2)
                    c1 = c1p.tile([128, QCOLS], dt.bfloat16, tag="c1")
                    ps = psA.tile([128, 2048], dt.float32, tag="psc", name="psc")
                    for kc in range(KCH):
                        kp = 128 if kc < KCH - 1 else IN - 128 * (KCH - 1)
                        lhsT = sb["w1t"][0:kp, kc * HR + mc * 128: kc * HR + (mc + 1) * 128]
                        for nt in range(4):
                            nc.tensor.matmul(
                                ps[:, nt * 512:nt * 512 + 500], lhsT,
                                xt[kc][0:kp, nt * 500:(nt + 1) * 500],
                                start=(kc == 0), stop=(kc == KCH - 1),
                            )
                    nc.scalar.activation(
                        c1[:].rearrange("p (nt c) -> p nt c", nt=4),
                        ps[:].rearrange("p (nt c) -> p nt c", nt=4)[:, :, 0:500],
                        Act.Identity, bias=sb["b1c"][:, mc:mc + 1], scale=1.0,
                    )
                    dst = (dacc[hh][0] if j == 0
                           else dtmpp.tile([128, QCOLS], dt.bfloat16,
                                           tag="dtmp", name="dtmp"))
                    for brx in range(QCOLS // T):
                        nc.vector.tensor_tensor_scan(
                            dst[:, brx * T:(brx + 1) * T],
                            sb["bz1"][:, mc, :],
                            c1[:, brx * T:(brx + 1) * T],
                            0.0, op0=Alu.mult, op1=Alu.add,
                        )
                    if j > 0:
                        src_acc = dacc[hh][j - 1]
                        if j == 3:
                            outap = V[:, 0, hh, q * 8:(q + 1) * 8, 0:T]
                        else:
                            outap = dacc[hh][j][:].rearrange(
                                "p (b t) -> p b t", t=T)
                        nc.vector.tensor_tensor(
                            outap,
                            src_acc[:].rearrange("p (b t) -> p b t", t=T),
                            dst[:].rearrange("p (b t) -> p b t", t=T),
                            Alu.add,
                        )

        # ---- phase B: merged serial scan + layer-2 pipeline + readout mm ----
        with tc.tile_pool(name="cB", bufs=1) as cBp, \
             tc.tile_pool(name="n1", bufs=2) as np_, \
             tc.tile_pool(name="c2", bufs=10) as c2p, \
             tc.tile_pool(name="d2s", bufs=10) as d2p, \
             tc.tile_pool(name="da2", bufs=4) as da2p, \
             tc.tile_pool(name="ps2", bufs=2, space="PSUM") as ps2p, \
             tc.tile_pool(name="ps3", bufs=2, space="PSUM") as ps3p:
            for name in ("w2t", "b2c", "bz2", "brc", "wrt"):
                load_const(cBp, name)

            def msteps(tau0, ntau):
                for tau in range(tau0, tau0 + ntau):
                    slot = tau % (2 * TC)
                    sprev = (ZC[:] if tau == 0
                             else S[:, :, :, :, (tau - 1) % (2 * TC)])
                    n = np_.tile([128, 2, 2, B], dt.bfloat16, tag="n")
                    nc.vector.tensor_tensor(n[:], M[:], sprev, Alu.subtract)
                    g = np_.tile([128, 2, 2, B], dt.bfloat16, tag="g")
                    nc.vector.tensor_tensor(g[:], n[:], sb["abc"][:], Alu.mult)
                    nc.vector.tensor_tensor(M[:], g[:], V[:, :, :, :, tau], Alu.add)
                    nc.vector.tensor_scalar(
                        S[:, :, :, :, slot], M[:], 1.0, None, op0=Alu.is_gt)

            def layer2(k):
                base = (k * TC) % (2 * TC)
                d2s = []
                for mc in range(MCH):
                    c2 = c2p.tile([128, B, TC + 1], dt.bfloat16, tag="c2")
                    nc.scalar.copy(c2[:, :, 0:1], D2K[:, mc, :, :])
                    ps = ps2p.tile([128, 1024], dt.float32, tag="ps2", name="ps2")
                    for nt in range(2):
                        for kc in range(2):
                            rhs = S[:, 0, kc, nt * 16:(nt + 1) * 16,
                                    base:base + TC]
                            nc.tensor.matmul(
                                ps[:, nt * 512:nt * 512 + 400],
                                sb["w2t"][:, kc * HR + mc * 128: kc * HR + (mc + 1) * 128],
                                rhs, start=(kc == 0), stop=(kc == 1),
                            )
                    nc.scalar.activation(
                        c2[:].rearrange("p (g b) i -> p g b i", g=2)[:, :, :, 1:TC + 1],
                        ps[:].rearrange("p (g x) -> p g x", g=2)[:, :, 0:400]
                            .rearrange("p g (b t) -> p g b t", t=TC),
                        Act.Identity, bias=sb["b2c"][:, mc:mc + 1], scale=1.0,
                    )
                    d2 = d2p.tile([128, B, TC + 1], dt.bfloat16, tag="d2")
                    nc.vector.tensor_tensor_scan(
                        d2[:].rearrange("p b t -> p (b t)"),
                        sb["bz2"][:, mc, :],
                        c2[:].rearrange("p b t -> p (b t)"),
                        0.0, op0=Alu.mult, op1=Alu.add,
                    )
                    nc.vector.tensor_copy(D2K[:, mc, :, :], d2[:, :, TC:TC + 1])
                    d2s.append(d2)
                for hh in range(2):
                    ta = da2p.tile([128, B, TC], dt.bfloat16, tag="ta")
                    tb2 = da2p.tile([128, B, TC], dt.bfloat16, tag="tb")
                    nc.vector.tensor_tensor(
                        ta[:], d2s[hh][:, :, 1:], d2s[2 + hh][:, :, 1:], Alu.add)
                    nc.vector.tensor_tensor(
                        tb2[:], d2s[4 + hh][:, :, 1:], d2s[6 + hh][:, :, 1:], Alu.add)
                    nc.vector.tensor_tensor(
                        V[:, 1, hh, :, (k + 1) * TC:(k + 2) * TC],
                        ta[:], tb2[:], Alu.add)

            def readout(k):
                base = ((k + 1) * TC) % (2 * TC)
                col0 = k * TC * B
                for t0, tn in ((0, 13), (13, 12)):
                    ps3 = ps3p.tile([OUT, 32 * tn], dt.float32, tag="ps3")
                    for kc in range(2):
                        rhs = S[:, 1, kc, :, base + t0:base + t0 + tn].rearrange(
                            "p b t -> p t b")
                        nc.tensor.matmul(
                            ps3[:], sb["wrt"][:, kc * OUT:(kc + 1) * OUT], rhs,
                            start=(kc == 0), stop=(kc == 1),
                        )
                    nc.scalar.activation(
                        C3[:, col0 + t0 * B: col0 + (t0 + tn) * B], ps3[:],
                        Act.Identity, bias=sb["brc"][:], scale=1.0,
                    )

            for k in range(NCHUNK + 1):
                msteps(k * TC, TC)
                if k >= 1:
                    readout(k - 1)
                if k < NCHUNK:
                    layer2(k)

        # ---- phase C: softmax-sum readout ----
        with tc.tile_pool(name="mr", bufs=1) as mrp, \
             tc.tile_pool(name="sm", bufs=4) as smp, \
             tc.tile_pool(name="psT", bufs=4, space="PSUM") as psTp, \
             tc.tile_pool(name="psX", bufs=1, space="PSUM") as psXp:
            for name in ("arz", "esel", "idm"):
                load_const(mrp, name)
            MR = mrp.tile([OUT, B * T], dt.float32)
            c3v = C3[:].rearrange("p (t b) -> p t b", b=B)
            mrv = MR[:].rearrange("p (t b) -> p t b", b=B)
            for b in range(B):
                nc.vector.tensor_tensor_scan(
                    mrv[:, :, b], sb["arz"][:], c3v[:, :, b],
                    0.0, op0=Alu.mult, op1=Alu.add,
                )
            EX = mrp.tile([OUT, B * T], dt.float32)
            nc.scalar.activation(EX[:], MR[:], Act.Exp)

            psAcc = psXp.tile([B, OUT], dt.float32)
            nblk = (B * T + 127) // 128
            for i in range(1, nblk):
                w = min(128, B * T - i * 128)
                psT = psTp.tile([128, OUT], dt.float32, tag="psT")
                nc.tensor.transpose(psT[0:w, :], EX[:, i * 128:i * 128 + w],
                                    sb["idm"][:])
                rs = smp.tile([128, 1], dt.float32, tag="rs")
                nc.vector.tensor_reduce(rs[0:w, :], psT[0:w, :],
                                        axis=mybir.AxisListType.X, op=Alu.add)
                ri = smp.tile([128, 1], dt.float32, tag="ri")
                nc.vector.reciprocal(ri[0:w, :], rs[0:w, :])
                sm = smp.tile([128, OUT], dt.float32, tag="sm")
                nc.vector.tensor_scalar(sm[0:w, :], psT[0:w, :], ri[0:w, :],
                                        None, op0=Alu.mult)
                nc.tensor.matmul(psAcc[:], sb["esel"][0:w, :], sm[0:w, :],
                                 start=(i == 1), stop=(i == nblk - 1))
            accS = smp.tile([B, OUT], dt.float32, tag="acc")
            nc.scalar.copy(accS[:], psAcc[:])
            nc.scalar.dma_start(out=out[:], in_=accS[:])

    nc.compile()
    return nc


_NC_CACHE = {}


def _get_program(num_devices=NCORES):
    if num_devices not in _NC_CACHE:
        _NC_CACHE[num_devices] = _build_program(num_devices)
    return _NC_CACHE[num_devices]


def make_in_maps(x, consts):
    """Per-core input maps (x sharded over batch, constants replicated)."""
    xs = np.ascontiguousarray(x.astype(np.float32).reshape(NCORES, ROWS, IN))
    return [{"x": xs[c], **consts} for c in range(NCORES)]


def kernel(x, W1, b1, tau_n1, tau_m1, W2, b2, tau_n2, tau_m2, Wr, br, tau_mr):
    from concourse.bass_utils import run_bass_kernel_spmd

    consts = _prep_constants(W1, b1, tau_n1, tau_m1, W2, b2, tau_n2, tau_m2,
                             Wr, br, tau_mr)
    nc = _get_program()
    in_maps = make_in_maps(np.asarray(x), consts)
    res = run_bass_kernel_spmd(nc, in_maps, list(range(NCORES)))
    out = np.concatenate([res.results[c]["out"] for c in range(NCORES)], axis=0)
    return out.astype(np.float32)

